# revision 1
# baseline (speedup 1.0000x reference)
"""Trainium2 Bass kernel for nn_HPool histogram_binning.

Math: z[n,c] = sum_hw tanh(x) * coeff[c, bin(x)] with 32 uniform bins over
[min(x), max(x)] (global).  Rewritten via cumulative-threshold form to avoid
any gather:
    coeff[c, b] = A_c + sum_{j=1..31} d[c,j] * [b >= j]
    z[n,c]      = A_c * T[n,c] + sum_j d[c,j] * S_j[n,c]
    T[n,c]      = sum_hw tanh(x)
    S_j[n,c]    = sum_hw tanh(x) * [x >= tau_j]     (tau_j = gmin + j*range/32)
Each S_j is one fused compare+mult+reduce (scalar_tensor_tensor) pass.

Sharding: data-parallel over N across 8 cores (8 samples each).
"""

import os
import numpy as np

N, C, H, W, BINS = 64, 64, 128, 128, 32
HW = H * W
NCORES = 8
NPC = N // NCORES          # samples per core
ROWS = NPC * C             # 512 rows per core, row r = n_local*C + c
P = 128
NT = ROWS // P             # 4 row-tiles
F = 2048                   # free-dim chunk
NF = HW // F               # 8 chunks per row-tile
NCHUNK = NT * NF

LAST_EXEC_NS = None
_CACHE = {}
import os as _os
NBINS_ACTIVE = int(_os.environ.get("KERNEL_NBINS", str(BINS - 1)))  # STT passes to emit
FP16 = bool(int(_os.environ.get("KERNEL_FP16", "0")))

# Engine assignment per bin j in 1..31 (rest on DVE). Tuned via cost model.
GP_BINS = ()                        # Pool can't run TensorScalarPtr (walrus check)
ACT_BINS = tuple(range(1, 12))      # scalar-engine relu/sign bins
VCOLS = 64                          # V layout: 0=T, 1..31=S/R, 32..62=G, 63=const


def _new_nc():
    import concourse.bacc as bacc

    return bacc.Bacc(
        "TRN2", target_bir_lowering=False, debug=False, num_devices=NCORES
    )


def _build_main():
    """Main kernel: thresholds are a [P, 31] input, z is the [ROWS, 1] output."""
    import concourse.mybir as mybir
    from concourse.tile import TileContext

    fp32 = mybir.dt.float32
    fp16 = mybir.dt.float16
    cdt = fp16 if FP16 else fp32
    AX = mybir.AxisListType.X
    OP = mybir.AluOpType

    nc = _new_nc()
    xs = nc.dram_tensor("xs", [ROWS, HW], fp32, kind="ExternalInput")
    dA = nc.dram_tensor("dA", [P, VCOLS], fp32, kind="ExternalInput")
    thi = nc.dram_tensor("th", [P, BINS - 1], fp32, kind="ExternalInput")
    ntt = nc.dram_tensor("ntt", [P, BINS - 1], fp32, kind="ExternalInput")  # -tanh(tau)
    nth = nc.dram_tensor("nth", [P, BINS - 1], fp32, kind="ExternalInput")  # -tau
    z = nc.dram_tensor("z", [ROWS, 1], fp32, kind="ExternalOutput")

    with TileContext(nc, num_cores=NCORES) as tc:
        with (
            tc.tile_pool(name="xp", bufs=4) as xp,
            tc.tile_pool(name="tp", bufs=2) as tp,
            tc.tile_pool(name="sp", bufs=2) as sp,
            tc.tile_pool(name="stat", bufs=1) as stat,
        ):
            dAs = stat.tile([P, VCOLS], fp32, tag="dAs")
            nc.sync.dma_start(out=dAs[:], in_=dA[:, :])
            th = stat.tile([P, BINS - 1], fp32, tag="th")
            nc.sync.dma_start(out=th[:], in_=thi[:, :])
            ntts = stat.tile([P, BINS - 1], fp32, tag="ntts")
            nc.sync.dma_start(out=ntts[:], in_=ntt[:, :])
            nths = stat.tile([P, BINS - 1], fp32, tag="nths")
            nc.sync.dma_start(out=nths[:], in_=nth[:, :])
            if FP16:
                thh = stat.tile([P, BINS - 1], fp16, tag="thh")
                nc.vector.tensor_copy(out=thh[:], in_=th[:])
            else:
                thh = th

            for t in range(NT):
                S = sp.tile([P, 2 * (BINS - 1) * NF], fp32, tag="S")
                TA = sp.tile([P, NF], fp32, tag="TA")
                for f in range(NF):
                    X = xp.tile([P, F], fp32, tag="X")
                    nc.sync.dma_start(
                        out=X[:], in_=xs[t * P:(t + 1) * P, f * F:(f + 1) * F]
                    )
                    T = tp.tile([P, F], cdt, tag="T")
                    nc.scalar.activation(
                        out=T[:], in_=X[:],
                        func=mybir.ActivationFunctionType.Tanh,
                        accum_out=TA[:, f:f + 1],
                    )
                    if FP16:
                        Xh = tp.tile([P, F], fp16, tag="Xh")
                        nc.scalar.copy(out=Xh[:], in_=X[:])
                    else:
                        Xh = X
                    SC = tp.tile([P, F], cdt, tag="SC")
                    if GP_BINS:
                        SCG = tp.tile([P, F], cdt, tag="SCG")
                    else:
                        SCG = None
                    SA = tp.tile([P, F], fp32, tag="SA")
                    SB = tp.tile([P, F], fp32, tag="SB")
                    for j in range(1, NBINS_ACTIVE + 1):
                        sacc = S[:, (j - 1) * NF + f:(j - 1) * NF + f + 1]
                        if j in ACT_BINS:
                            nc.scalar.activation(
                                out=SA[:], in_=T[:],
                                func=mybir.ActivationFunctionType.Relu,
                                bias=ntts[:, j - 1:j], accum_out=sacc,
                            )
                            gacc = S[:, ((BINS - 1) + (j - 1)) * NF + f:
                                     ((BINS - 1) + (j - 1)) * NF + f + 1]
                            nc.scalar.activation(
                                out=SB[:], in_=X[:],
                                func=mybir.ActivationFunctionType.Sign,
                                bias=nths[:, j - 1:j], accum_out=gacc,
                            )
                            continue
                        eng = nc.gpsimd if j in GP_BINS else nc.vector
                        out_t = SCG if j in GP_BINS else SC
                        eng.scalar_tensor_tensor(
                            out=out_t[:], in0=Xh[:], scalar=thh[:, j - 1:j], in1=T[:],
                            op0=OP.is_ge, op1=OP.mult,
                            accum_out=sacc,
                        )
                V = sp.tile([P, VCOLS], fp32, tag="V")
                nc.vector.memset(V[:], 0.0)
                nc.vector.tensor_reduce(out=V[:, 0:1], in_=TA[:], axis=AX, op=OP.add)
                for j in range(1, NBINS_ACTIVE + 1):
                    nc.vector.tensor_reduce(
                        out=V[:, j:j + 1], in_=S[:, (j - 1) * NF:j * NF],
                        axis=AX, op=OP.add,
                    )
                    if j in ACT_BINS:
                        nc.vector.tensor_reduce(
                            out=V[:, 31 + j:32 + j],
                            in_=S[:, ((BINS - 1) + (j - 1)) * NF:
                                   ((BINS - 1) + j) * NF],
                            axis=AX, op=OP.add,
                        )
                nc.vector.memset(V[:, 63:64], 1.0)
                ZC = sp.tile([P, VCOLS], fp32, tag="ZC")
                zcol = sp.tile([P, 1], fp32, tag="zcol")
                nc.vector.tensor_tensor(out=ZC[:], in0=V[:], in1=dAs[:], op=OP.mult)
                nc.vector.tensor_reduce(out=zcol[:], in_=ZC[:], axis=AX, op=OP.add)
                nc.sync.dma_start(out=z[t * P:(t + 1) * P, :], in_=zcol[:])
    nc.compile()
    return nc


def _prep_in_maps(x: np.ndarray, coeff: np.ndarray):
    gmin = np.float32(x.min())
    gmax = np.float32(x.max())
    step = np.float32((gmax - gmin) * np.float32(1.0 / 32.0))
    js = np.arange(1, BINS, dtype=np.float32)
    taus = (gmin + js * step).astype(np.float32)        # tau_1..tau_31
    th128 = np.ascontiguousarray(np.tile(taus, (P, 1)), dtype=np.float32)

    tanh_tau = np.tanh(taus.astype(np.float64)).astype(np.float32)
    ntt128 = np.ascontiguousarray(np.tile(-tanh_tau, (P, 1)), dtype=np.float32)
    nth128 = np.ascontiguousarray(np.tile(-taus, (P, 1)), dtype=np.float32)

    d64 = np.diff(coeff, axis=1)                     # d_j, j=1..31  [64,31]
    W64 = np.zeros((C, VCOLS), dtype=np.float64)
    W64[:, 0] = coeff[:, 0]                          # A_c * T
    W64[:, 1:32] = d64                               # d_j * (S_j or R_j)
    const = np.zeros(C, dtype=np.float64)
    for j in ACT_BINS:
        tt = np.float64(tanh_tau[j - 1])
        W64[:, 32 + j - 1] = d64[:, j - 1] * tt / 2.0      # d_j*tt*G_j/2
        const += d64[:, j - 1] * tt * (HW / 2.0)           # d_j*tt*HW/2
    W64[:, 63] = const
    dA128 = np.ascontiguousarray(np.tile(W64.astype(np.float32), (2, 1)))

    xr = x.reshape(N, C, HW)
    in_maps = []
    for k in range(NCORES):
        shard = np.ascontiguousarray(
            xr[k * NPC:(k + 1) * NPC].reshape(ROWS, HW), dtype=np.float32
        )
        in_maps.append({"xs": shard, "dA": dA128, "th": th128,
                        "ntt": ntt128, "nth": nth128})
    return in_maps


def kernel(x: np.ndarray, coeff: np.ndarray) -> np.ndarray:
    global LAST_EXEC_NS
    from concourse.bass_utils import run_bass_kernel_spmd

    x = np.asarray(x, dtype=np.float32)
    coeff = np.asarray(coeff, dtype=np.float32)

    if "nc" not in _CACHE:
        _CACHE["nc"] = _build_main()
    nc = _CACHE["nc"]

    in_maps = _prep_in_maps(x, coeff)

    trace = bool(os.environ.get("KERNEL_TRACE"))
    res = run_bass_kernel_spmd(
        nc, in_maps, list(range(NCORES)), trace=trace,
    )
    LAST_EXEC_NS = res.exec_time_ns

    out = np.empty((N, C), dtype=np.float32)
    for k in range(NCORES):
        out[k * NPC:(k + 1) * NPC] = res.results[k]["z"].reshape(NPC, C)
    return out



# revision 8
# speedup vs baseline: 2.3222x; 2.3222x over previous
"""Trainium2 Bass kernel for nn_HPool histogram_binning.

Math: z[n,c] = sum_hw tanh(x) * coeff[c, bin(x)] with 32 uniform bins over
[min(x), max(x)] (global min/max, computed host-side like the thresholds).

Scheme ("hinge + count stats at 4x"):
  T = tanh(x) (fp16, scalar engine, fused row-accum gives sum(T)).
  For each interior bin edge tau_j (j=1..31), with tt_j = tanh(tau_j):
    count stat  G_j = sum_f [T >= tt_j]          (one tensor_scalar, 4x mode)
    hinge stat  R_j = sum_f relu(T - tt_j)       (one tensor_scalar, 4x mode)
  Exact recovery: S_{>=j} := sum_f T*[T >= tt_j] = R_j + tt_j * G_j, and the
  per-bin tanh-mass S_b is a difference of adjacent S_{>=}.
  Tail trick: for bins fully outside |x| <= XCUT, tanh saturates so
  S_b ~= sign(bin) * cnt_b (error ~1e-3 of z); hinges are only emitted for
  the ~18 central edges. Counts are emitted for all 31 edges.
  z[r] is then a per-row linear mix of the ~50 raw stats with host-computed
  per-channel weights (single tensor_tensor mult + reduce per row-tile).

Cost: ~50 stats/element instead of 32 full passes; DVE tensor_scalar with
immediate scalars + accum_out runs in 4x perf mode (0.25 cyc/elem, fp16),
with ~11 count stats offloaded to the scalar engine (Sign+bias+accum) to
balance ACT (tanh pass) and DVE.

Sharding: data-parallel over N across 8 cores (8 samples each).
"""

import os
import numpy as np

N, C, H, W, BINS = 64, 64, 128, 128, 32
HW = H * W
NCORES = 8
NPC = N // NCORES          # samples per core
ROWS = NPC * C             # 512 rows per core, row r = n_local*C + c
P = 128
NT = ROWS // P             # 4 row-tiles
F = 8192                   # free-dim chunk (half a row-tile)
NF = HW // F               # 2 chunks per row-tile

XCUT = float(os.environ.get("KERNEL_XCUT", "3.0"))   # hinge edges kept where |tau| <= XCUT
N_ACT = int(os.environ.get("KERNEL_NACT", "11"))     # count stats on scalar engine

LAST_EXEC_NS = None
_CACHE = {}


def _edge_info(gmin: float, gmax: float):
    """Edges tau_1..tau_31, tanh thresholds, hinge set, ACT/DVE count split."""
    step = (np.float64(gmax) - np.float64(gmin)) / np.float64(BINS)
    edges = (np.float64(gmin) + step * np.arange(1, BINS)).astype(np.float64)
    tt = np.tanh(edges)
    jh = [j for j in range(BINS - 1) if abs(edges[j]) <= XCUT]
    assert jh and jh == list(range(jh[0], jh[-1] + 1)), "hinge edges not contiguous"
    act_j = list(range(min(N_ACT, BINS - 1)))        # count edges on ACT (Sign)
    return edges, tt, jh, set(act_j)


def _stat_cols(jh):
    """Column layout inside the [P, 64] stats tile (per half, offset 0/64)."""
    rcol = {j: 1 + i for i, j in enumerate(jh)}           # hinge stats
    g0 = 1 + len(jh)
    gcol = {j: g0 + j for j in range(BINS - 1)}           # count stats
    assert g0 + BINS - 1 <= 63
    return rcol, gcol                                      # col 0 = sum(T), 63 = const


def _host_weights(coeff: np.ndarray, gmin: float, gmax: float):
    """Per-channel mixing weights over the raw stat columns (fp64 -> fp32)."""
    edges, tt, jh, act_j = _edge_info(gmin, gmax)
    rcol, gcol = _stat_cols(jh)
    jhset = set(jh)
    tau_lo = np.float64(gmin) + (np.float64(gmax) - np.float64(gmin)) / BINS * np.arange(BINS)

    w = np.zeros((C, 64), dtype=np.float64)
    const = np.zeros(C, dtype=np.float64)

    def add_g(j, v):
        if j in act_j:   # raw stat is sum(sign(T-tt)) = 2G - n
            w[:, gcol[j]] += v / 2.0
            const[:] += v * (HW / 2.0)
        else:            # raw stat is G directly
            w[:, gcol[j]] += v

    def add_s_geq(e, v):
        # S_{>=e} = M_j + tt_j*G_j - tt_j*n  (M_j = sum max(T, tt_j))
        if e == 0:
            w[:, 0] += v                     # sum(T)
        elif e < BINS:
            j = e - 1
            w[:, rcol[j]] += v
            add_g(j, v * tt[j])
            const[:] += -v * tt[j] * HW
        # e == BINS: zero

    def add_g_geq(e, v):
        if e == 0:
            const[:] += v * HW
        elif e < BINS:
            add_g(e - 1, v)

    for b in range(BINS):
        wb = coeff[:, b].astype(np.float64)
        lo_ok = (b == 0) or (b - 1) in jhset
        hi_ok = (b == BINS - 1) or b in jhset
        if lo_ok and hi_ok:
            add_s_geq(b, wb)
            add_s_geq(b + 1, -wb)
        else:
            sgn = 1.0 if tau_lo[b] >= 0 else -1.0
            add_g_geq(b, wb * sgn)
            add_g_geq(b + 1, -wb * sgn)

    w[:, 63] = const
    return w.astype(np.float32)


def _new_nc():
    import concourse.bacc as bacc

    return bacc.Bacc(
        "TRN2", target_bir_lowering=False, debug=False, num_devices=NCORES
    )


def _build_main(gmin: float, gmax: float):
    import concourse.mybir as mybir
    from concourse.tile import TileContext

    fp32 = mybir.dt.float32
    fp16 = mybir.dt.float16
    AX = mybir.AxisListType.X
    OP = mybir.AluOpType
    AF = mybir.ActivationFunctionType

    edges, tt, jh, act_j = _edge_info(gmin, gmax)
    rcol, gcol = _stat_cols(jh)
    dve_count_j = [j for j in range(BINS - 1) if j not in act_j]

    nc = _new_nc()
    xs = nc.dram_tensor("xs", [ROWS, HW], fp32, kind="ExternalInput")
    wt = nc.dram_tensor("wt", [P, 64], fp32, kind="ExternalInput")
    bs = nc.dram_tensor("bs", [P, max(len(act_j), 1)], fp32, kind="ExternalInput")
    z = nc.dram_tensor("z", [ROWS, 1], fp32, kind="ExternalOutput")

    with TileContext(nc, num_cores=NCORES) as tc:
        with (
            tc.tile_pool(name="xp", bufs=2) as xp,
            tc.tile_pool(name="tp", bufs=2) as tp,
            tc.tile_pool(name="sp", bufs=2) as sp,
            tc.tile_pool(name="stat", bufs=1) as stat,
        ):
            wts = stat.tile([P, 64], fp32, tag="wts")
            nc.sync.dma_start(out=wts[:], in_=wt[:, :])
            bss = stat.tile([P, max(len(act_j), 1)], fp32, tag="bss")
            nc.sync.dma_start(out=bss[:], in_=bs[:, :])

            for t in range(NT):
                V = sp.tile([P, 128], fp32, tag="V")
                nc.vector.memset(V[:], 0.0)
                for h in range(NF):
                    off = 64 * h
                    X = xp.tile([P, F], fp32, tag="X")
                    nc.sync.dma_start(
                        out=X[:], in_=xs[t * P:(t + 1) * P, h * F:(h + 1) * F]
                    )
                    T = tp.tile([P, F], fp16, tag="T")
                    nc.scalar.activation(
                        out=T[:], in_=X[:], func=AF.Tanh,
                        accum_out=V[:, off:off + 1],
                    )
                    SA = tp.tile([P, F], fp16, tag="SA")
                    for i, j in enumerate(sorted(act_j)):
                        nc.scalar.activation(
                            out=SA[:], in_=T[:], func=AF.Sign,
                            bias=bss[:, i:i + 1],
                            accum_out=V[:, off + gcol[j]:off + gcol[j] + 1],
                        )
                    # With accum_out, op1 is the REDUCTION op: accum = reduce_op1(op0(in, s1)).
                    SD = tp.tile([P, F], fp16, tag="SD")
                    for j in jh:
                        nc.vector.tensor_scalar(
                            out=SD[:], in0=T[:],
                            scalar1=float(tt[j]), scalar2=0.0,
                            op0=OP.max, op1=OP.add,
                            accum_out=V[:, off + rcol[j]:off + rcol[j] + 1],
                        )
                    for j in dve_count_j:
                        nc.vector.tensor_scalar(
                            out=SD[:], in0=T[:],
                            scalar1=float(tt[j]), scalar2=0.0,
                            op0=OP.is_ge, op1=OP.add,
                            accum_out=V[:, off + gcol[j]:off + gcol[j] + 1],
                        )
                Vs = sp.tile([P, 64], fp32, tag="Vs")
                nc.vector.tensor_tensor(
                    out=Vs[:], in0=V[:, 0:64], in1=V[:, 64:128], op=OP.add
                )
                nc.vector.memset(Vs[:, 63:64], 1.0)
                ZC = sp.tile([P, 64], fp32, tag="ZC")
                nc.vector.tensor_tensor(out=ZC[:], in0=Vs[:], in1=wts[:], op=OP.mult)
                zcol = sp.tile([P, 1], fp32, tag="zcol")
                nc.vector.tensor_reduce(out=zcol[:], in_=ZC[:], axis=AX, op=OP.add)
                nc.sync.dma_start(out=z[t * P:(t + 1) * P, :], in_=zcol[:])
    nc.compile()
    return nc


def _prep_in_maps(x: np.ndarray, coeff: np.ndarray, gmin: float, gmax: float):
    wt = _host_weights(coeff, gmin, gmax)                 # [C, 64]
    wt128 = np.ascontiguousarray(wt[np.arange(P) % C])    # row r -> channel r%64

    _, tt, _, act_j = _edge_info(gmin, gmax)
    aj = sorted(act_j)
    nbias = max(len(aj), 1)
    bs128 = np.zeros((P, nbias), dtype=np.float32)
    for i, j in enumerate(aj):
        bs128[:, i] = np.float32(-tt[j])

    xr = x.reshape(N, C, HW)
    in_maps = []
    for k in range(NCORES):
        shard = np.ascontiguousarray(
            xr[k * NPC:(k + 1) * NPC].reshape(ROWS, HW), dtype=np.float32
        )
        in_maps.append({"xs": shard, "wt": wt128, "bs": bs128})
    return in_maps


def kernel(x: np.ndarray, coeff: np.ndarray) -> np.ndarray:
    global LAST_EXEC_NS
    from concourse.bass_utils import run_bass_kernel_spmd

    x = np.asarray(x, dtype=np.float32)
    coeff = np.asarray(coeff, dtype=np.float32)

    gmin = float(x.min())
    gmax = float(x.max())

    key = ("nc", gmin, gmax)
    if key not in _CACHE:
        _CACHE[key] = _build_main(gmin, gmax)
    nc = _CACHE[key]
    _CACHE["nc"] = nc   # test.py reads _CACHE["nc"] for the cost-model timeline

    in_maps = _prep_in_maps(x, coeff, gmin, gmax)

    trace = bool(os.environ.get("KERNEL_TRACE"))
    res = run_bass_kernel_spmd(
        nc, in_maps, list(range(NCORES)), trace=trace,
    )
    LAST_EXEC_NS = res.exec_time_ns

    out = np.empty((N, C), dtype=np.float32)
    for k in range(NCORES):
        out[k * NPC:(k + 1) * NPC] = res.results[k]["z"].reshape(NPC, C)
    return out
